# revision 24
# baseline (speedup 1.0000x reference)
"""Trainium2 Bass kernel for nn_DEQLatentSpaceOpt (DDIM trajectory DEQ iteration).

Composite restructure: the whole 3-iteration recursion is linear in x, so it
collapses to
    y[t] = sum_s A3[t,s]*C^3(x[s])                    (trajectory term)
         + sum_{a<3} A_a[t,0]*C^a(xT)                 (xT basis, host conv)
         + sum_{a<3,j} B_a[t,j]*C^a(e_j)              (temb basis, host conv)
with A/B coefficient matrices built on the host by propagating the recursion.
Device work per core (125 trajectory images): three chained 3x3 convs on
TensorE (9 shifted block-diag matmuls, bf16), then ONE triangular combine.
Cross-core coupling: A3's cross-core blocks are exactly rank-3, so each core
contributes 3 weighted aggregate images, exchanged in a single 8-rank
AllGather and folded into the combine's carry matmul.

vs. the iterative baseline this removes 2 of 3 triangular passes, all
per-iteration totals/carry matmuls, the temb bias path, and the two
iteration-barrier AllGathers (516K -> 393K PE columns).
"""

import numpy as np
import ml_dtypes

import jax
import concourse.bacc as bacc
import concourse.mybir as mybir
import concourse.tile as tile
from concourse.bass_interp import get_hw_module
from concourse import bass2jax

BF16 = mybir.dt.bfloat16
F32 = mybir.dt.float32
FP8 = mybir.dt.float8e4

# conv tap pairs for fp8 DoubleRow matmuls: (tapA, tapB, free-offset delta);
# tap index ti = 3*(dy+1) + (dx+1), flat free offset = dy*66 + dx in the
# row-padded conv layout. Pair 4 is tap 8 alone (zero second k-tile).
TAP_PAIRS = [(0, 1, 1), (2, 3, 64), (4, 5, 1), (6, 7, 1), (8, None, -1)]

N_CORES = 8
T = 1000
C = 3
HW = 4096  # 64*64
TLOC = T // N_CORES  # 125 rows per core
G = 42  # partition groups; partition p = 3g + c, 126 used of 128
S = 3  # image slots per partition (42*3 = 126 slots >= 125 images)

# padded image layout per partition: row stride 66 (1 left pad + 64 px + 1
# right pad), one 66-wide gap row between images, one lead gap row.
ROWS = S * 65 + 1  # 196
RW = 66
TAPS = [(dy, dx) for dy in (-1, 0, 1) for dx in (-1, 0, 1)]
CHUNK_ROWS = 8  # conv matmul chunk: 8 image rows x 64 px = 512 cols
NCH = 64 // CHUNK_ROWS  # 8 chunks per image slot
PS_GRP = 2  # psum tile holds 2 chunks (1024 f32 = 2 banks)
NAGG = 3  # rank of cross-core coupling
NCARRY = 9 * N_CORES + 9 + 27  # 108 carry rhs rows

_compiled = None


def _build_module(sim_mode=False):
    """sim_mode: single-core variant with the AllGather replaced by
    byte-equivalent local DMAs, for TimelineSim cost estimation only."""
    nc = bacc.Bacc(
        "TRN2",
        target_bir_lowering=False,
        debug=False,
        num_devices=1 if sim_mode else N_CORES,
    )

    # I/O
    x_arr = nc.dram_tensor("x_arr", [128, S, HW], BF16, kind="ExternalInput").ap()
    w9 = nc.dram_tensor("w9", [9, 128, 128], BF16, kind="ExternalInput").ap()
    triw = nc.dram_tensor("triw", [9, 128, 128], BF16, kind="ExternalInput").ap()
    cxw = nc.dram_tensor("cxw", [S, 128, 128], BF16, kind="ExternalInput").ap()
    totw = nc.dram_tensor("totw", [S, 128, 3 * NAGG], BF16, kind="ExternalInput").ap()
    basw = nc.dram_tensor("basw", [128 - 9 * N_CORES, HW], BF16, kind="ExternalInput").ap()
    out_arr = nc.dram_tensor("out_arr", [128, S, HW], BF16, kind="ExternalOutput").ap()

    TRI_IDX = {(j, l): 3 * j + l for j in range(S) for l in range(S)}

    with tile.TileContext(nc) as tc:
        with (
            tc.tile_pool(name="persist", bufs=1) as pp,
            tc.tile_pool(name="pconv", bufs=2, space="PSUM") as pconv,
            tc.tile_pool(name="pmisc", bufs=2, space="PSUM") as pmisc,
            tc.tile_pool(name="dram", bufs=2, space="DRAM") as dp,
        ):
            # persistent tiles
            convA = pp.tile([128, ROWS, RW], BF16, tag="convA")
            convB = pp.tile([128, ROWS, RW], BF16, tag="convB")
            stag = pp.tile([128, S, HW], BF16, tag="stag")
            e = pp.tile([128, S, HW], BF16, tag="e")
            rhs_cx = pp.tile([128, HW], BF16, tag="rhs_cx")
            agin_s = pp.tile([3 * NAGG, HW], BF16, tag="agin_s")
            w9s = pp.tile([128, 9, 128], BF16, tag="w9s")
            tris = pp.tile([128, 9, 128], BF16, tag="tris")
            cxs = pp.tile([128, S, 128], BF16, tag="cxs")
            tots = pp.tile([128, S, 3 * NAGG], BF16, tag="tots")

            # convA: zero only pads (data fully overwritten by the x DMA,
            # which also covers partitions 126/127). convB: zero everything —
            # evacs only write partitions 0:126, and garbage bits in 126/127
            # would be read (x0 weight) by the next conv's matmuls, where a
            # NaN pattern poisons the accumulation.
            nc.gpsimd.memset(convA[:, :, 0:66:65], 0.0)  # x pads
            for gr in range(0, ROWS, 65):  # lead + inter-image gap rows
                nc.gpsimd.memset(convA[:, gr], 0.0)
            nc.gpsimd.memset(convB[:], 0.0)
            # load conv weights in one DMA (first matmul waits on this)
            nc.sync.dma_start(w9s[:], w9.rearrange("i p m -> p i m"))

            # load x (already bf16, host-quantized) straight into the padded
            # conv-input layout, in slot-quarters so early conv matmuls can
            # start while later pieces still load
            NH = 4
            for j in range(S):
                r0 = 1 + 65 * j
                for h in range(NH):
                    hw2 = HW // NH
                    rh = 64 // NH
                    nc.sync.dma_start(
                        convA[:, r0 + rh * h : r0 + rh * (h + 1), 1:65],
                        x_arr[:, j, h * hw2 : (h + 1) * hw2].rearrange(
                            "p (a b) -> p a b", b=64
                        ),
                    )

            nc.sync.dma_start(tris[:], triw.rearrange("i p m -> p i m"))
            for j in range(S):
                nc.sync.dma_start(cxs[:, j], cxw[j])
                nc.sync.dma_start(tots[:, j], totw[j])
            nc.sync.dma_start(rhs_cx[9 * N_CORES : 128, :], basw[:])

            # ---- three chained convs: convA -> convB -> convA -> e ----
            for st in range(3):
                src = convA if st % 2 == 0 else convB
                dst = convB if st % 2 == 0 else convA
                last = st == 2
                for cg in range(NCH // PS_GRP):  # chunk groups of 2
                    c0 = cg * PS_GRP * 512
                    for j in range(S):
                        r0 = 1 + 65 * j
                        pt = pconv.tile([128, PS_GRP * 512], F32, tag="pconv")
                        for ci in range(PS_GRP):
                            ch = cg * PS_GRP + ci
                            rr = r0 + ch * CHUNK_ROWS
                            for ti, (dy, dx) in enumerate(TAPS):
                                nc.tensor.matmul(
                                    pt[:, ci * 512 : (ci + 1) * 512],
                                    w9s[:, ti],
                                    src[
                                        :,
                                        rr + dy : rr + CHUNK_ROWS + dy,
                                        1 + dx : 65 + dx,
                                    ],
                                    start=(ti == 0),
                                    stop=(ti == 8),
                                )
                        # evac psum -> bf16; alternate ACT/DVE engines
                        use_act = (j * (NCH // PS_GRP) + cg) % 2 == 0
                        if last:
                            if use_act:
                                nc.scalar.activation(
                                    e[:, j, c0 : c0 + PS_GRP * 512],
                                    pt[:],
                                    mybir.ActivationFunctionType.Copy,
                                )
                            else:
                                nc.vector.tensor_copy(
                                    e[:, j, c0 : c0 + PS_GRP * 512], pt[:]
                                )
                        else:
                            rows = PS_GRP * CHUNK_ROWS
                            rr = 1 + 65 * j + cg * rows
                            if use_act:
                                nc.scalar.activation(
                                    dst[0:126, rr : rr + rows, 1:65],
                                    pt[0:126].rearrange("p (a b) -> p a b", b=64),
                                    mybir.ActivationFunctionType.Copy,
                                )
                            else:
                                nc.vector.tensor_copy(
                                    dst[0:126, rr : rr + rows, 1:65],
                                    pt[0:126].rearrange("p (a b) -> p a b", b=64),
                                )
                    if last:
                        # aggregates for this column group (all 3 ranks at
                        # once; out rows 3i+c)
                        for ci in range(PS_GRP):
                            ch = cg * PS_GRP + ci
                            ptt = pmisc.tile([3 * NAGG, 512], F32, tag="pmisc")
                            for l in range(S):
                                nc.tensor.matmul(
                                    ptt[:],
                                    tots[:, l],
                                    e[:, l, ch * 512 : (ch + 1) * 512],
                                    start=(l == 0),
                                    stop=(l == S - 1),
                                )
                            nc.vector.tensor_copy(
                                agin_s[:, ch * 512 : (ch + 1) * 512], ptt[:]
                            )
                        # AllGather in two column-halves so the first half's
                        # DMA+collective chain hides under the second half of
                        # the stage-3 convs, and the second half's chain hides
                        # under the first half's combine groups — the carry
                        # matmuls then never stall the PE.
                        if cg in (1, NCH // PS_GRP - 1):
                            hf = 0 if cg == 1 else 1
                            h0 = hf * (HW // 2)
                            ag_in = dp.tile(
                                [3 * NAGG, HW // 2], BF16, tag=f"ag_in{hf}"
                            )
                            ag_out = dp.tile(
                                [N_CORES * 3 * NAGG, HW // 2],
                                BF16,
                                tag=f"ag_out{hf}",
                            )
                            nc.sync.dma_start(
                                ag_in[:], agin_s[:, h0 : h0 + HW // 2]
                            )
                            if sim_mode:
                                for r in range(N_CORES):
                                    nc.sync.dma_start(
                                        ag_out[3 * NAGG * r : 3 * NAGG * (r + 1), :],
                                        ag_in[:],
                                    )
                            else:
                                nc.gpsimd.collective_compute(
                                    "AllGather",
                                    mybir.AluOpType.bypass,
                                    replica_groups=[list(range(N_CORES))],
                                    ins=[ag_in.opt()],
                                    outs=[ag_out.opt()],
                                )
                            nc.sync.dma_start(
                                rhs_cx[0 : 9 * N_CORES, h0 : h0 + HW // 2],
                                ag_out[:],
                            )

            # ---- single composite combine: tri + carry matmuls ----
            # cg-major so the first groups only need the first AllGather half
            for cg in range(NCH // PS_GRP):
                for j in range(S):
                    pc = pmisc.tile([128, PS_GRP * 512], F32, tag="pmisc")
                    for ci in range(PS_GRP):
                        c0 = (cg * PS_GRP + ci) * 512
                        sl = slice(ci * 512, (ci + 1) * 512)
                        # tri matmuls first (no AllGather dependency — they
                        # overlap the collective), carry last
                        for l in range(S):
                            nc.tensor.matmul(
                                pc[:, sl],
                                tris[:, TRI_IDX[(j, l)]],
                                e[:, l, c0 : c0 + 512],
                                start=(l == 0),
                                stop=False,
                            )
                        nc.tensor.matmul(
                            pc[:, sl],
                            cxs[:, j],
                            rhs_cx[:, c0 : c0 + 512],
                            start=False,
                            stop=True,
                        )
                    c0 = cg * PS_GRP * 512
                    if (j * (NCH // PS_GRP) + cg) % 2 == 1:
                        nc.scalar.activation(
                            stag[:, j, c0 : c0 + PS_GRP * 512],
                            pc[:],
                            mybir.ActivationFunctionType.Copy,
                        )
                    else:
                        nc.vector.tensor_copy(
                            stag[:, j, c0 : c0 + PS_GRP * 512], pc[:]
                        )
                    # stream this chunk out while later chunks compute
                    nc.sync.dma_start(
                        out_arr[:, j, c0 : c0 + PS_GRP * 512],
                        stag[:, j, c0 : c0 + PS_GRP * 512],
                    )

    nc.compile()
    nc.m = get_hw_module(nc.m)
    return nc


def _conv_np(img, w):
    """SAME zero-pad correlation, img [C,64,64], w [co,ci,3,3]."""
    pad = np.pad(img, ((0, 0), (1, 1), (1, 1)))
    out = np.zeros_like(img)
    for ky in range(3):
        for kx in range(3):
            out += np.einsum(
                "oi,ihw->ohw", w[:, :, ky, kx], pad[:, ky : ky + 64, kx : kx + 64]
            )
    return out


def _build_inputs(x, alpha_ratio, et_coeff, et_prevsum_coeff, conv_w, temb, t):
    """Host-side composite-coefficient precompute; returns per-core in_maps."""
    ar_ = np.asarray(alpha_ratio, np.float64).reshape(T)
    etc_ = np.asarray(et_coeff, np.float64).reshape(T)
    epc_ = np.asarray(et_prevsum_coeff, np.float64).reshape(T)
    temb = np.asarray(temb, np.float64)
    t = np.asarray(t).astype(np.int64)
    conv_w = np.asarray(conv_w, np.float64)
    x = np.asarray(x, np.float32)
    tembsel = temb[t]  # [T, C]

    bf = ml_dtypes.bfloat16

    # ---- coefficient propagation: y = sum_a A[a] C^a(x) + B[a] C^a(e_j) ----
    A = np.zeros((4, T + 1, T + 1))
    A[0] = np.eye(T + 1)
    B = np.zeros((4, T + 1, C))
    for _ in range(3):
        nA = np.zeros_like(A)
        nB = np.zeros_like(B)
        for a in range(3):
            nA[a + 1][1:] = epc_[:, None] * np.cumsum(etc_[:, None] * A[a][:T], axis=0)
            nB[a + 1][1:] = epc_[:, None] * np.cumsum(etc_[:, None] * B[a][:T], axis=0)
        nA[0][0, 0] = 1.0
        nA[0][1:, 0] += ar_
        nB[0][1:] = epc_[:, None] * np.cumsum(etc_[:, None] * tembsel, axis=0)
        A, B = nA, nB
    A3 = A[3]

    # ---- rank-3 factorization of the cross-core part of A3 ----
    blk = A3[7 * TLOC + 1 :, : 7 * TLOC]
    U, Sv, Vt = np.linalg.svd(blk, full_matrices=False)
    R = (Vt[:NAGG].T * np.sqrt(Sv[:NAGG]))  # [875, 3]
    L = np.zeros((T + 1, NAGG))
    for k in range(1, N_CORES):
        t0, t1 = k * TLOC + 1, (k + 1) * TLOC + 1
        sc = slice(0, k * TLOC)
        sol, *_ = np.linalg.lstsq(R[sc], A3[t0:t1, sc].T, rcond=None)
        L[t0:t1] = sol.T

    # ---- host conv basis images (rows beyond 36 stay zero: they pad the
    # carry rhs up to partition 128) ----
    basw = np.zeros((128 - 9 * N_CORES, HW))
    xT = x[0].astype(np.float64)
    cur = xT
    for a in range(3):
        basw[3 * a : 3 * a + 3] = cur.reshape(C, HW)
        cur = _conv_np(cur, conv_w)
    for jj in range(C):
        eimg = np.zeros((C, 64, 64))
        eimg[jj] = 1.0
        cur = eimg
        for a in range(3):
            basw[9 + 9 * a + 3 * jj : 9 + 9 * a + 3 * jj + 3] = cur.reshape(C, HW)
            cur = _conv_np(cur, conv_w)

    # shared: conv tap weights, block-diagonal [3g+ci, 3g+co]
    w9 = np.zeros((9, 128, 128), np.float32)
    for ti, (dy, dx) in enumerate(TAPS):
        blkw = conv_w[:, :, dy + 1, dx + 1].T  # [ci, co]
        for g in range(G):
            w9[ti, 3 * g : 3 * g + 3, 3 * g : 3 * g + 3] = blkw
    w9 = w9.astype(bf)

    gs = np.arange(G)
    in_maps = []
    for k in range(N_CORES):
        o = k * TLOC

        def valid_g(j):
            return gs[3 * gs + j <= TLOC - 1]

        def ocol(g, j):  # j=2 outputs shifted +3 partitions (host unpack)
            return 3 * (g + 1) if j == S - 1 else 3 * g

        tri = np.zeros((9, 128, 128), np.float32)
        for j in range(S):
            gj = valid_g(j)
            tj = o + 3 * gj + j + 1  # output rows
            oc = np.array([ocol(g, j) for g in gj])
            for l in range(S):
                gl = valid_g(l)
                sl_ = o + 3 * gl + l  # source rows
                vals = A3[np.ix_(tj, sl_)]  # [len_j, len_l]
                for c in range(C):
                    tri[3 * j + l][np.ix_(3 * gl + c, oc + c)] = vals.T

        cx = np.zeros((S, 128, 128), np.float32)
        for j in range(S):
            gj = valid_g(j)
            tj = o + 3 * gj + j + 1
            oc = np.array([ocol(g, j) for g in gj])
            for c in range(C):
                for m in range(k):  # earlier cores' aggregates
                    for i in range(NAGG):
                        cx[j, 9 * m + 3 * i + c, oc + c] = L[tj, i]
                for a in range(3):  # xT basis
                    cx[j, 9 * N_CORES + 3 * a + c, oc + c] = A[a][tj, 0]
                for a in range(3):  # temb basis
                    for jj in range(C):
                        cx[j, 9 * N_CORES + 9 + 9 * a + 3 * jj + c, oc + c] = B[a][
                            tj, jj
                        ]

        tot = np.zeros((S, 128, 3 * NAGG), np.float32)
        if k < N_CORES - 1:  # last core's aggregates are never consumed
            for l in range(S):
                gl = valid_g(l)
                sl_ = o + 3 * gl + l
                for i in range(NAGG):
                    for c in range(C):
                        tot[l, 3 * gl + c, 3 * i + c] = R[sl_, i]

        xa = np.zeros((128, S, HW), bf)
        for j in range(S):
            rows = o + 3 * gs + j  # x row index for slot (g, j); <= 1000
            xa[3 * gs[:, None] + np.arange(C), j] = x[rows].reshape(G, C, HW)

        in_maps.append(
            {
                "x_arr": xa,
                "w9": w9,
                "triw": tri.astype(bf),
                "cxw": cx.astype(bf),
                "totw": tot.astype(bf),
                "basw": basw.astype(bf),
            }
        )
    return in_maps


class _Runner:
    """Compile once, keep the jitted sharded executable for reuse."""

    def __init__(self):
        from jax.sharding import Mesh, PartitionSpec
        from jax.experimental.shard_map import shard_map

        self.nc = _build_module()
        nc = self.nc
        bass2jax.install_neuronx_cc_hook()

        part_name = (
            nc.partition_id_tensor.name if nc.partition_id_tensor else None
        )
        in_names, out_names, out_avals, zero_shapes = [], [], [], []
        for alloc in nc.m.functions[0].allocations:
            if not isinstance(alloc, mybir.MemoryLocationSet):
                continue
            name = alloc.memorylocations[0].name
            if alloc.kind == "ExternalInput":
                if name != part_name:
                    in_names.append(name)
            elif alloc.kind == "ExternalOutput":
                out_names.append(name)
                shape = tuple(alloc.tensor_shape)
                dtype = mybir.dt.np(alloc.dtype)
                out_avals.append(jax.core.ShapedArray(shape, dtype))
                zero_shapes.append((shape, dtype))
        n_params = len(in_names)
        n_outs = len(out_names)
        all_names = in_names + out_names
        if part_name is not None:
            all_names = all_names + [part_name]
        self.in_names = in_names
        self.out_names = out_names
        self.n_params = n_params
        self.zero_shapes = zero_shapes

        def _body(*args):
            operands = list(args)
            if part_name is not None:
                operands.append(bass2jax.partition_id_tensor())
            outs = bass2jax._bass_exec_p.bind(
                *operands,
                out_avals=tuple(out_avals),
                in_names=tuple(all_names),
                out_names=tuple(out_names),
                lowering_input_output_aliases=(),
                sim_require_finite=True,
                sim_require_nnan=True,
                nc=nc,
            )
            return tuple(outs)

        devices = jax.devices()[:N_CORES]
        mesh = Mesh(np.asarray(devices), ("core",))
        in_specs = (PartitionSpec("core"),) * (n_params + n_outs)
        out_specs = (PartitionSpec("core"),) * n_outs
        self.fn = jax.jit(
            shard_map(
                _body, mesh=mesh, in_specs=in_specs, out_specs=out_specs,
                check_rep=False,
            ),
            donate_argnums=tuple(range(n_params, n_params + n_outs)),
            keep_unused=True,
        )

    def __call__(self, in_maps):
        concat_in = [
            np.concatenate([np.asarray(m[name]) for m in in_maps], axis=0)
            for name in self.in_names
        ]
        zeros = [
            np.zeros((N_CORES * s[0], *s[1:]), d) for s, d in self.zero_shapes
        ]
        outs = self.fn(*concat_in, *zeros)
        return [
            {
                name: np.asarray(outs[i]).reshape(N_CORES, -1, *outs[i].shape[1:])[c]
                for i, name in enumerate(self.out_names)
            }
            for c in range(N_CORES)
        ]


def kernel(x, t, alpha_ratio, et_coeff, et_prevsum_coeff, conv_w, temb):
    global _compiled
    if _compiled is None:
        _compiled = _Runner()

    in_maps = _build_inputs(x, alpha_ratio, et_coeff, et_prevsum_coeff, conv_w, temb, t)
    results = _compiled(in_maps)

    x = np.asarray(x, np.float32)
    y = np.empty((T + 1, C, 64, 64), np.float32)
    y[0] = x[0]
    gs = np.arange(G)
    for k in range(N_CORES):
        o = k * TLOC
        oa = results[k]["out_arr"]  # [128, S, HW]
        for j in range(S):
            gv = gs[3 * gs + j <= TLOC - 1]
            if j == S - 1:
                # shifted layout: partition group g+1 holds image 3g+2
                gp = gv + 1
                rows = o + 3 * gp  # = o + (3g+2) + 1
                y[rows] = oa[(3 * gp[:, None] + np.arange(C)), j].reshape(
                    len(gp), C, 64, 64
                )
            else:
                rows = o + 3 * gv + j + 1
                y[rows] = oa[(3 * gv[:, None] + np.arange(C)), j].reshape(
                    len(gv), C, 64, 64
                )
    return y


# revision 25
# speedup vs baseline: 1.0248x; 1.0248x over previous
"""Trainium2 Bass kernel for nn_DEQLatentSpaceOpt (DDIM trajectory DEQ iteration).

Composite restructure: the whole 3-iteration recursion is linear in x, so it
collapses to
    y[t] = sum_s A3[t,s]*C^3(x[s])                    (trajectory term)
         + sum_{a<3} A_a[t,0]*C^a(xT)                 (xT basis, host conv)
         + sum_{a<3,j} B_a[t,j]*C^a(e_j)              (temb basis, host conv)
with A/B coefficient matrices built on the host by propagating the recursion.
Device work per core (125 trajectory images): three chained 3x3 convs on
TensorE (9 shifted block-diag matmuls, bf16), then ONE triangular combine.
Cross-core coupling: A3's cross-core blocks are exactly rank-3, so each core
contributes 3 weighted aggregate images, exchanged in a single 8-rank
AllGather and folded into the combine's carry matmul.

vs. the iterative baseline this removes 2 of 3 triangular passes, all
per-iteration totals/carry matmuls, the temb bias path, and the two
iteration-barrier AllGathers (516K -> 393K PE columns).
"""

import numpy as np
import ml_dtypes

import jax
import concourse.bacc as bacc
import concourse.mybir as mybir
import concourse.tile as tile
from concourse.bass_interp import get_hw_module
from concourse import bass2jax

BF16 = mybir.dt.bfloat16
F32 = mybir.dt.float32
FP8 = mybir.dt.float8e4

# conv tap pairs for fp8 DoubleRow matmuls: (tapA, tapB, free-offset delta);
# tap index ti = 3*(dy+1) + (dx+1), flat free offset = dy*66 + dx in the
# row-padded conv layout. Pair 4 is tap 8 alone (zero second k-tile).
TAP_PAIRS = [(0, 1, 1), (2, 3, 64), (4, 5, 1), (6, 7, 1), (8, None, -1)]

N_CORES = 8
T = 1000
C = 3
HW = 4096  # 64*64
TLOC = T // N_CORES  # 125 rows per core
G = 42  # partition groups; partition p = 3g + c, 126 used of 128
S = 3  # image slots per partition (42*3 = 126 slots >= 125 images)

# padded image layout per partition: row stride 66 (1 left pad + 64 px + 1
# right pad), one 66-wide gap row between images, one lead gap row.
ROWS = S * 65 + 1  # 196
RW = 66
TAPS = [(dy, dx) for dy in (-1, 0, 1) for dx in (-1, 0, 1)]
CHUNK_ROWS = 8  # conv matmul chunk: 8 image rows x 64 px = 512 cols
NCH = 64 // CHUNK_ROWS  # 8 chunks per image slot
PS_GRP = 2  # psum tile holds 2 chunks (1024 f32 = 2 banks)
NAGG = 3  # rank of cross-core coupling
NCARRY = 9 * N_CORES + 9 + 27  # 108 carry rhs rows

_compiled = None


def _build_module(sim_mode=False):
    """sim_mode: single-core variant with the AllGather replaced by
    byte-equivalent local DMAs, for TimelineSim cost estimation only."""
    nc = bacc.Bacc(
        "TRN2",
        target_bir_lowering=False,
        debug=False,
        num_devices=1 if sim_mode else N_CORES,
    )

    # I/O
    x_arr = nc.dram_tensor("x_arr", [128, S, HW], BF16, kind="ExternalInput").ap()
    w9 = nc.dram_tensor("w9", [9, 128, 128], BF16, kind="ExternalInput").ap()
    triw = nc.dram_tensor("triw", [9, 128, 128], BF16, kind="ExternalInput").ap()
    cxw = nc.dram_tensor("cxw", [S, 128, 128], BF16, kind="ExternalInput").ap()
    totw = nc.dram_tensor("totw", [S, 128, 3 * NAGG], BF16, kind="ExternalInput").ap()
    basw = nc.dram_tensor("basw", [128 - 9 * N_CORES, HW], BF16, kind="ExternalInput").ap()
    out_arr = nc.dram_tensor("out_arr", [128, S, HW], F32, kind="ExternalOutput").ap()

    TRI_IDX = {(j, l): 3 * j + l for j in range(S) for l in range(S)}

    with tile.TileContext(nc) as tc:
        with (
            tc.tile_pool(name="persist", bufs=1) as pp,
            tc.tile_pool(name="pconv", bufs=2, space="PSUM") as pconv,
            tc.tile_pool(name="pmisc", bufs=2, space="PSUM") as pmisc,
            tc.tile_pool(name="dram", bufs=2, space="DRAM") as dp,
        ):
            # persistent tiles
            convA = pp.tile([128, ROWS, RW], BF16, tag="convA")
            convB = pp.tile([128, ROWS, RW], BF16, tag="convB")
            stag = pp.tile([128, S, HW], F32, tag="stag")
            e = pp.tile([128, S, HW], BF16, tag="e")
            rhs_cx = pp.tile([128, HW], BF16, tag="rhs_cx")
            agin_s = pp.tile([3 * NAGG, HW], BF16, tag="agin_s")
            w9s = pp.tile([128, 9, 128], BF16, tag="w9s")
            tris = pp.tile([128, 9, 128], BF16, tag="tris")
            cxs = pp.tile([128, S, 128], BF16, tag="cxs")
            tots = pp.tile([128, S, 3 * NAGG], BF16, tag="tots")

            # convA: zero only pads (data fully overwritten by the x DMA,
            # which also covers partitions 126/127). convB: zero everything —
            # evacs only write partitions 0:126, and garbage bits in 126/127
            # would be read (x0 weight) by the next conv's matmuls, where a
            # NaN pattern poisons the accumulation.
            nc.gpsimd.memset(convA[:, :, 0:66:65], 0.0)  # x pads
            for gr in range(0, ROWS, 65):  # lead + inter-image gap rows
                nc.gpsimd.memset(convA[:, gr], 0.0)
            nc.gpsimd.memset(convB[:], 0.0)
            # load weights
            for i in range(9):
                nc.sync.dma_start(w9s[:, i], w9[i])

            # load x (already bf16, host-quantized) straight into the padded
            # conv-input layout, in slot-quarters so early conv matmuls can
            # start while later pieces still load
            NH = 4
            for j in range(S):
                r0 = 1 + 65 * j
                for h in range(NH):
                    hw2 = HW // NH
                    rh = 64 // NH
                    nc.sync.dma_start(
                        convA[:, r0 + rh * h : r0 + rh * (h + 1), 1:65],
                        x_arr[:, j, h * hw2 : (h + 1) * hw2].rearrange(
                            "p (a b) -> p a b", b=64
                        ),
                    )

            for i in range(9):
                nc.sync.dma_start(tris[:, i], triw[i])
            for j in range(S):
                nc.sync.dma_start(cxs[:, j], cxw[j])
                nc.sync.dma_start(tots[:, j], totw[j])
            nc.sync.dma_start(rhs_cx[9 * N_CORES : 128, :], basw[:])

            # ---- three chained convs: convA -> convB -> convA -> e ----
            for st in range(3):
                src = convA if st % 2 == 0 else convB
                dst = convB if st % 2 == 0 else convA
                last = st == 2
                for cg in range(NCH // PS_GRP):  # chunk groups of 2
                    c0 = cg * PS_GRP * 512
                    for j in range(S):
                        r0 = 1 + 65 * j
                        pt = pconv.tile([128, PS_GRP * 512], F32, tag="pconv")
                        for ci in range(PS_GRP):
                            ch = cg * PS_GRP + ci
                            rr = r0 + ch * CHUNK_ROWS
                            for ti, (dy, dx) in enumerate(TAPS):
                                nc.tensor.matmul(
                                    pt[:, ci * 512 : (ci + 1) * 512],
                                    w9s[:, ti],
                                    src[
                                        :,
                                        rr + dy : rr + CHUNK_ROWS + dy,
                                        1 + dx : 65 + dx,
                                    ],
                                    start=(ti == 0),
                                    stop=(ti == 8),
                                )
                        # evac psum -> bf16; alternate ACT/DVE engines
                        use_act = (j * (NCH // PS_GRP) + cg) % 2 == 0
                        if last:
                            if use_act:
                                nc.scalar.activation(
                                    e[:, j, c0 : c0 + PS_GRP * 512],
                                    pt[:],
                                    mybir.ActivationFunctionType.Copy,
                                )
                            else:
                                nc.vector.tensor_copy(
                                    e[:, j, c0 : c0 + PS_GRP * 512], pt[:]
                                )
                        else:
                            rows = PS_GRP * CHUNK_ROWS
                            rr = 1 + 65 * j + cg * rows
                            if use_act:
                                nc.scalar.activation(
                                    dst[0:126, rr : rr + rows, 1:65],
                                    pt[0:126].rearrange("p (a b) -> p a b", b=64),
                                    mybir.ActivationFunctionType.Copy,
                                )
                            else:
                                nc.vector.tensor_copy(
                                    dst[0:126, rr : rr + rows, 1:65],
                                    pt[0:126].rearrange("p (a b) -> p a b", b=64),
                                )
                    if last:
                        # aggregates for this column group (all 3 ranks at
                        # once; out rows 3i+c)
                        for ci in range(PS_GRP):
                            ch = cg * PS_GRP + ci
                            ptt = pmisc.tile([3 * NAGG, 512], F32, tag="pmisc")
                            for l in range(S):
                                nc.tensor.matmul(
                                    ptt[:],
                                    tots[:, l],
                                    e[:, l, ch * 512 : (ch + 1) * 512],
                                    start=(l == 0),
                                    stop=(l == S - 1),
                                )
                            nc.vector.tensor_copy(
                                agin_s[:, ch * 512 : (ch + 1) * 512], ptt[:]
                            )
                        # AllGather in two column-halves so the first half's
                        # DMA+collective chain hides under the second half of
                        # the stage-3 convs, and the second half's chain hides
                        # under the first half's combine groups — the carry
                        # matmuls then never stall the PE.
                        if cg in (1, NCH // PS_GRP - 1):
                            hf = 0 if cg == 1 else 1
                            h0 = hf * (HW // 2)
                            ag_in = dp.tile(
                                [3 * NAGG, HW // 2], BF16, tag=f"ag_in{hf}"
                            )
                            ag_out = dp.tile(
                                [N_CORES * 3 * NAGG, HW // 2],
                                BF16,
                                tag=f"ag_out{hf}",
                            )
                            nc.sync.dma_start(
                                ag_in[:], agin_s[:, h0 : h0 + HW // 2]
                            )
                            if sim_mode:
                                for r in range(N_CORES):
                                    nc.sync.dma_start(
                                        ag_out[3 * NAGG * r : 3 * NAGG * (r + 1), :],
                                        ag_in[:],
                                    )
                            else:
                                nc.gpsimd.collective_compute(
                                    "AllGather",
                                    mybir.AluOpType.bypass,
                                    replica_groups=[list(range(N_CORES))],
                                    ins=[ag_in.opt()],
                                    outs=[ag_out.opt()],
                                )
                            nc.sync.dma_start(
                                rhs_cx[0 : 9 * N_CORES, h0 : h0 + HW // 2],
                                ag_out[:],
                            )

            # ---- single composite combine: tri + carry matmuls ----
            # cg-major so the first groups only need the first AllGather half
            for cg in range(NCH // PS_GRP):
                for j in range(S):
                    pc = pmisc.tile([128, PS_GRP * 512], F32, tag="pmisc")
                    for ci in range(PS_GRP):
                        c0 = (cg * PS_GRP + ci) * 512
                        sl = slice(ci * 512, (ci + 1) * 512)
                        # tri matmuls first (no AllGather dependency — they
                        # overlap the collective), carry last
                        for l in range(S):
                            nc.tensor.matmul(
                                pc[:, sl],
                                tris[:, TRI_IDX[(j, l)]],
                                e[:, l, c0 : c0 + 512],
                                start=(l == 0),
                                stop=False,
                            )
                        nc.tensor.matmul(
                            pc[:, sl],
                            cxs[:, j],
                            rhs_cx[:, c0 : c0 + 512],
                            start=False,
                            stop=True,
                        )
                    c0 = cg * PS_GRP * 512
                    if (j * (NCH // PS_GRP) + cg) % 2 == 1:
                        nc.scalar.activation(
                            stag[:, j, c0 : c0 + PS_GRP * 512],
                            pc[:],
                            mybir.ActivationFunctionType.Copy,
                        )
                    else:
                        nc.vector.tensor_copy(
                            stag[:, j, c0 : c0 + PS_GRP * 512], pc[:]
                        )
                    # stream this chunk out while later chunks compute
                    nc.sync.dma_start(
                        out_arr[:, j, c0 : c0 + PS_GRP * 512],
                        stag[:, j, c0 : c0 + PS_GRP * 512],
                    )

    nc.compile()
    nc.m = get_hw_module(nc.m)
    return nc


def _conv_np(img, w):
    """SAME zero-pad correlation, img [C,64,64], w [co,ci,3,3]."""
    pad = np.pad(img, ((0, 0), (1, 1), (1, 1)))
    out = np.zeros_like(img)
    for ky in range(3):
        for kx in range(3):
            out += np.einsum(
                "oi,ihw->ohw", w[:, :, ky, kx], pad[:, ky : ky + 64, kx : kx + 64]
            )
    return out


def _build_inputs(x, alpha_ratio, et_coeff, et_prevsum_coeff, conv_w, temb, t):
    """Host-side composite-coefficient precompute; returns per-core in_maps."""
    ar_ = np.asarray(alpha_ratio, np.float64).reshape(T)
    etc_ = np.asarray(et_coeff, np.float64).reshape(T)
    epc_ = np.asarray(et_prevsum_coeff, np.float64).reshape(T)
    temb = np.asarray(temb, np.float64)
    t = np.asarray(t).astype(np.int64)
    conv_w = np.asarray(conv_w, np.float64)
    x = np.asarray(x, np.float32)
    tembsel = temb[t]  # [T, C]

    bf = ml_dtypes.bfloat16

    # ---- coefficient propagation: y = sum_a A[a] C^a(x) + B[a] C^a(e_j) ----
    A = np.zeros((4, T + 1, T + 1))
    A[0] = np.eye(T + 1)
    B = np.zeros((4, T + 1, C))
    for _ in range(3):
        nA = np.zeros_like(A)
        nB = np.zeros_like(B)
        for a in range(3):
            nA[a + 1][1:] = epc_[:, None] * np.cumsum(etc_[:, None] * A[a][:T], axis=0)
            nB[a + 1][1:] = epc_[:, None] * np.cumsum(etc_[:, None] * B[a][:T], axis=0)
        nA[0][0, 0] = 1.0
        nA[0][1:, 0] += ar_
        nB[0][1:] = epc_[:, None] * np.cumsum(etc_[:, None] * tembsel, axis=0)
        A, B = nA, nB
    A3 = A[3]

    # ---- rank-3 factorization of the cross-core part of A3 ----
    blk = A3[7 * TLOC + 1 :, : 7 * TLOC]
    U, Sv, Vt = np.linalg.svd(blk, full_matrices=False)
    R = (Vt[:NAGG].T * np.sqrt(Sv[:NAGG]))  # [875, 3]
    L = np.zeros((T + 1, NAGG))
    for k in range(1, N_CORES):
        t0, t1 = k * TLOC + 1, (k + 1) * TLOC + 1
        sc = slice(0, k * TLOC)
        sol, *_ = np.linalg.lstsq(R[sc], A3[t0:t1, sc].T, rcond=None)
        L[t0:t1] = sol.T

    # ---- host conv basis images (rows beyond 36 stay zero: they pad the
    # carry rhs up to partition 128) ----
    basw = np.zeros((128 - 9 * N_CORES, HW))
    xT = x[0].astype(np.float64)
    cur = xT
    for a in range(3):
        basw[3 * a : 3 * a + 3] = cur.reshape(C, HW)
        cur = _conv_np(cur, conv_w)
    for jj in range(C):
        eimg = np.zeros((C, 64, 64))
        eimg[jj] = 1.0
        cur = eimg
        for a in range(3):
            basw[9 + 9 * a + 3 * jj : 9 + 9 * a + 3 * jj + 3] = cur.reshape(C, HW)
            cur = _conv_np(cur, conv_w)

    # shared: conv tap weights, block-diagonal [3g+ci, 3g+co]
    w9 = np.zeros((9, 128, 128), np.float32)
    for ti, (dy, dx) in enumerate(TAPS):
        blkw = conv_w[:, :, dy + 1, dx + 1].T  # [ci, co]
        for g in range(G):
            w9[ti, 3 * g : 3 * g + 3, 3 * g : 3 * g + 3] = blkw
    w9 = w9.astype(bf)

    gs = np.arange(G)
    in_maps = []
    for k in range(N_CORES):
        o = k * TLOC

        def valid_g(j):
            return gs[3 * gs + j <= TLOC - 1]

        def ocol(g, j):  # j=2 outputs shifted +3 partitions (host unpack)
            return 3 * (g + 1) if j == S - 1 else 3 * g

        tri = np.zeros((9, 128, 128), np.float32)
        for j in range(S):
            gj = valid_g(j)
            tj = o + 3 * gj + j + 1  # output rows
            oc = np.array([ocol(g, j) for g in gj])
            for l in range(S):
                gl = valid_g(l)
                sl_ = o + 3 * gl + l  # source rows
                vals = A3[np.ix_(tj, sl_)]  # [len_j, len_l]
                for c in range(C):
                    tri[3 * j + l][np.ix_(3 * gl + c, oc + c)] = vals.T

        cx = np.zeros((S, 128, 128), np.float32)
        for j in range(S):
            gj = valid_g(j)
            tj = o + 3 * gj + j + 1
            oc = np.array([ocol(g, j) for g in gj])
            for c in range(C):
                for m in range(k):  # earlier cores' aggregates
                    for i in range(NAGG):
                        cx[j, 9 * m + 3 * i + c, oc + c] = L[tj, i]
                for a in range(3):  # xT basis
                    cx[j, 9 * N_CORES + 3 * a + c, oc + c] = A[a][tj, 0]
                for a in range(3):  # temb basis
                    for jj in range(C):
                        cx[j, 9 * N_CORES + 9 + 9 * a + 3 * jj + c, oc + c] = B[a][
                            tj, jj
                        ]

        tot = np.zeros((S, 128, 3 * NAGG), np.float32)
        if k < N_CORES - 1:  # last core's aggregates are never consumed
            for l in range(S):
                gl = valid_g(l)
                sl_ = o + 3 * gl + l
                for i in range(NAGG):
                    for c in range(C):
                        tot[l, 3 * gl + c, 3 * i + c] = R[sl_, i]

        xa = np.zeros((128, S, HW), bf)
        for j in range(S):
            rows = o + 3 * gs + j  # x row index for slot (g, j); <= 1000
            xa[3 * gs[:, None] + np.arange(C), j] = x[rows].reshape(G, C, HW)

        in_maps.append(
            {
                "x_arr": xa,
                "w9": w9,
                "triw": tri.astype(bf),
                "cxw": cx.astype(bf),
                "totw": tot.astype(bf),
                "basw": basw.astype(bf),
            }
        )
    return in_maps


class _Runner:
    """Compile once, keep the jitted sharded executable for reuse."""

    def __init__(self):
        from jax.sharding import Mesh, PartitionSpec
        from jax.experimental.shard_map import shard_map

        self.nc = _build_module()
        nc = self.nc
        bass2jax.install_neuronx_cc_hook()

        part_name = (
            nc.partition_id_tensor.name if nc.partition_id_tensor else None
        )
        in_names, out_names, out_avals, zero_shapes = [], [], [], []
        for alloc in nc.m.functions[0].allocations:
            if not isinstance(alloc, mybir.MemoryLocationSet):
                continue
            name = alloc.memorylocations[0].name
            if alloc.kind == "ExternalInput":
                if name != part_name:
                    in_names.append(name)
            elif alloc.kind == "ExternalOutput":
                out_names.append(name)
                shape = tuple(alloc.tensor_shape)
                dtype = mybir.dt.np(alloc.dtype)
                out_avals.append(jax.core.ShapedArray(shape, dtype))
                zero_shapes.append((shape, dtype))
        n_params = len(in_names)
        n_outs = len(out_names)
        all_names = in_names + out_names
        if part_name is not None:
            all_names = all_names + [part_name]
        self.in_names = in_names
        self.out_names = out_names
        self.n_params = n_params
        self.zero_shapes = zero_shapes

        def _body(*args):
            operands = list(args)
            if part_name is not None:
                operands.append(bass2jax.partition_id_tensor())
            outs = bass2jax._bass_exec_p.bind(
                *operands,
                out_avals=tuple(out_avals),
                in_names=tuple(all_names),
                out_names=tuple(out_names),
                lowering_input_output_aliases=(),
                sim_require_finite=True,
                sim_require_nnan=True,
                nc=nc,
            )
            return tuple(outs)

        devices = jax.devices()[:N_CORES]
        mesh = Mesh(np.asarray(devices), ("core",))
        in_specs = (PartitionSpec("core"),) * (n_params + n_outs)
        out_specs = (PartitionSpec("core"),) * n_outs
        self.fn = jax.jit(
            shard_map(
                _body, mesh=mesh, in_specs=in_specs, out_specs=out_specs,
                check_rep=False,
            ),
            donate_argnums=tuple(range(n_params, n_params + n_outs)),
            keep_unused=True,
        )

    def __call__(self, in_maps):
        concat_in = [
            np.concatenate([np.asarray(m[name]) for m in in_maps], axis=0)
            for name in self.in_names
        ]
        zeros = [
            np.zeros((N_CORES * s[0], *s[1:]), d) for s, d in self.zero_shapes
        ]
        outs = self.fn(*concat_in, *zeros)
        return [
            {
                name: np.asarray(outs[i]).reshape(N_CORES, -1, *outs[i].shape[1:])[c]
                for i, name in enumerate(self.out_names)
            }
            for c in range(N_CORES)
        ]


def kernel(x, t, alpha_ratio, et_coeff, et_prevsum_coeff, conv_w, temb):
    global _compiled
    if _compiled is None:
        _compiled = _Runner()

    in_maps = _build_inputs(x, alpha_ratio, et_coeff, et_prevsum_coeff, conv_w, temb, t)
    results = _compiled(in_maps)

    x = np.asarray(x, np.float32)
    y = np.empty((T + 1, C, 64, 64), np.float32)
    y[0] = x[0]
    gs = np.arange(G)
    for k in range(N_CORES):
        o = k * TLOC
        oa = results[k]["out_arr"]  # [128, S, HW]
        for j in range(S):
            gv = gs[3 * gs + j <= TLOC - 1]
            if j == S - 1:
                # shifted layout: partition group g+1 holds image 3g+2
                gp = gv + 1
                rows = o + 3 * gp  # = o + (3g+2) + 1
                y[rows] = oa[(3 * gp[:, None] + np.arange(C)), j].reshape(
                    len(gp), C, 64, 64
                )
            else:
                rows = o + 3 * gv + j + 1
                y[rows] = oa[(3 * gv[:, None] + np.arange(C)), j].reshape(
                    len(gv), C, 64, 64
                )
    return y
